# revision 1
# baseline (speedup 1.0000x reference)
"""nn_CrossAttention Trainium2 kernel — 8-core SPMD Bass/Tile implementation.

Sharding: core p -> batch b = p//2, query-row half h = p%2 (data parallel over
B=4, sequence-parallel over TN within each batch pair).

Per-core dataflow:
  tT,xT   PE-transpose of the f32 inputs (feature dim onto partitions)
  qT,kT   projections in channel-major layout (f32r = tf32-class precision)
  v       projection in natural row-major layout (bf16)
  sT      score tiles computed *transposed* (keys on partitions, queries free)
  eT      exp(SCALE*s) straight from PSUM via ScalarE, cast to bf16
  D       softmax denominators via ones-vector matmul (PE partition reduce)
  oT      v.T @ eT accumulated over keys, normalized by 1/D broadcast
  AllGather within core pairs exchanges oT halves; the reference's
  "transpose(1,2).reshape" permutation then becomes *contiguous* DRAM rows
  (z-buffer), so the output projection reads it with plain DMAs.
  out     permuted-o chunks @ Wp + bp  (full TN rows; host keeps its half)
"""
from contextlib import ExitStack

import numpy as np

import concourse.bass as bass
import concourse.tile as tile
from concourse import bacc, mybir
from concourse.bass_utils import run_bass_kernel_spmd
from concourse.masks import make_identity

F32 = mybir.dt.float32
BF16 = mybir.dt.bfloat16
F32R = mybir.dt.float32r
EXP = mybir.ActivationFunctionType.Exp

B, N, TN, C = 4, 4096, 4096, 384
TNS = TN // 2
SCALE = (C // 8) ** -0.5
N_CORES = 8

QK_DT = F32R   # q/k/score path (tf32-class)
VE_DT = BF16   # v/e/o path


def build(qk_dt=QK_DT, ve_dt=VE_DT, repeat=1, with_collective=True):
    nc = bacc.Bacc("TRN2", target_bir_lowering=False, debug=False,
                   num_devices=N_CORES)
    x_d = nc.dram_tensor("x", [N, C], F32, kind="ExternalInput").ap()
    t_d = nc.dram_tensor("t", [TNS, C], F32, kind="ExternalInput").ap()
    w_d = {n: nc.dram_tensor(n, [C, C], F32, kind="ExternalInput").ap()
           for n in ("Wq", "Wk", "Wv", "Wp")}
    bp_d = nc.dram_tensor("bp", [1, C], F32, kind="ExternalInput").ap()
    out_d = nc.dram_tensor("out", [TN, C], F32, kind="ExternalOutput").ap()

    with tile.TileContext(nc) as tc:
        _kernel_body(nc, tc, x_d, t_d, w_d, bp_d, out_d, qk_dt, ve_dt,
                     repeat, with_collective)
    nc.compile()
    return nc


def _kernel_body(nc, tc, x_d, t_d, w_d, bp_d, out_d, qk_st, ve_st,
                 repeat, with_collective):
    with ExitStack() as ctx:
        consts = ctx.enter_context(tc.tile_pool(name="consts", bufs=1))
        persist = ctx.enter_context(tc.tile_pool(name="persist", bufs=1))
        dram = ctx.enter_context(tc.tile_pool(name="dram", bufs=1, space="DRAM"))

        ident = consts.tile([128, 128], F32)
        make_identity(nc, ident)
        ones_col = consts.tile([128, 1], ve_st)
        nc.vector.memset(ones_col[:], 1.0)
        ones_row = consts.tile([1, 128], F32)
        nc.vector.memset(ones_row[:], 1.0)

        w_sb = {}
        with tc.tile_pool(name="wstage", bufs=2) as wstage:
            for name in ("Wq", "Wk", "Wv", "Wp"):
                cw = persist.tile([128, 3 * C], qk_st, name=f"{name}_sb",
                                  tag=f"{name}_sb")
                for dc in range(3):
                    st = wstage.tile([128, C], F32, name="wst", tag="wst")
                    nc.sync.dma_start(st[:], w_d[name][dc * 128:(dc + 1) * 128, :])
                    nc.scalar.copy(cw[:, dc * C:(dc + 1) * C], st[:])
                w_sb[name] = cw
            bst = wstage.tile([1, C], F32, name="bst", tag="wst")
            nc.sync.dma_start(bst[:], bp_d[:])
            with tc.tile_pool(name="bpsum", bufs=1, space="PSUM") as bpsum:
                bias_ps = bpsum.tile([128, C], F32)
                nc.tensor.matmul(bias_ps[:], ones_row[:], bst[:],
                                 start=True, stop=True)
                bias_b = persist.tile([128, C], F32)
                nc.vector.tensor_copy(bias_b[:], bias_ps[:])

        def wch(name, dc, cc=None):
            if cc is None:
                return w_sb[name][:, dc * C:(dc + 1) * C]
            return w_sb[name][:, dc * C + cc * 128: dc * C + (cc + 1) * 128]

        for rep in range(repeat):
            _one_pass(nc, tc, x_d, t_d, out_d, qk_st, ve_st, ident, ones_col,
                      ones_row, wch, bias_b, dram, with_collective, rep)


def _one_pass(nc, tc, x_d, t_d, out_d, qk_st, ve_st, ident, ones_col,
              ones_row, wch, bias_b, dram, with_collective, rep):
    with tc.tile_pool(name="attin", bufs=1) as attin:
        # ---- tT -> qT ----
        with tc.tile_pool(name="tstage", bufs=3) as tstage, \
             tc.tile_pool(name="trpsum", bufs=2, space="PSUM") as trpsum:
            tT = [tstage.tile([128, TNS], qk_st, name=f"tT{dc}", tag=f"tT{dc}",
                              bufs=1) for dc in range(3)]
            for i in range(TNS // 128):
                trow = tstage.tile([128, C], F32, name="trow", tag="trow")
                nc.sync.dma_start(trow[:], t_d[i * 128:(i + 1) * 128, :])
                for dc in range(3):
                    pst = trpsum.tile([128, 128], F32, name="pst", tag="pst")
                    nc.tensor.transpose(pst[:], trow[:, dc * 128:(dc + 1) * 128],
                                        ident[:])
                    nc.vector.tensor_copy(tT[dc][:, i * 128:(i + 1) * 128], pst[:])
            qT = attin.tile([128, 3 * TNS], qk_st, name="qT", tag="qT")
            with tc.tile_pool(name="qpsum", bufs=2, space="PSUM") as qpsum:
                for cc in range(3):
                    for nt in range(TNS // 512):
                        ps = qpsum.tile([128, 512], F32, name="qps", tag="qps")
                        for dc in range(3):
                            nc.tensor.matmul(
                                ps[:], wch("Wq", dc, cc),
                                tT[dc][:, nt * 512:(nt + 1) * 512],
                                start=(dc == 0), stop=(dc == 2))
                        nc.scalar.copy(
                            qT[:, cc * TNS + nt * 512: cc * TNS + (nt + 1) * 512],
                            ps[:])

        # ---- xT -> kT, v ----
        with tc.tile_pool(name="xstage", bufs=3) as xstage, \
             tc.tile_pool(name="xtrpsum", bufs=2, space="PSUM") as xtrpsum:
            xT = [xstage.tile([128, N], qk_st, name=f"xT{dc}", tag=f"xT{dc}",
                              bufs=1) for dc in range(3)]
            for i in range(N // 128):
                xrow = xstage.tile([128, C], F32, name="xrow", tag="xrow")
                nc.sync.dma_start(xrow[:], x_d[i * 128:(i + 1) * 128, :])
                for dc in range(3):
                    pst = xtrpsum.tile([128, 128], F32, name="xpst", tag="xpst")
                    nc.tensor.transpose(pst[:], xrow[:, dc * 128:(dc + 1) * 128],
                                        ident[:])
                    nc.vector.tensor_copy(xT[dc][:, i * 128:(i + 1) * 128], pst[:])
            kT = [attin.tile([128, N], qk_st, name=f"kT{cc}", tag=f"kT{cc}")
                  for cc in range(3)]
            v_all = attin.tile([128, 32 * C], ve_st, name="v_all", tag="v_all")
            with tc.tile_pool(name="kvpsum", bufs=3, space="PSUM") as kvpsum:
                for cc in range(3):
                    for nt in range(N // 512):
                        ps = kvpsum.tile([128, 512], F32, name="kps", tag="kps")
                        for dc in range(3):
                            nc.tensor.matmul(
                                ps[:], wch("Wk", dc, cc),
                                xT[dc][:, nt * 512:(nt + 1) * 512],
                                start=(dc == 0), stop=(dc == 2))
                        nc.scalar.copy(kT[cc][:, nt * 512:(nt + 1) * 512], ps[:])
                for n32 in range(32):
                    ps = kvpsum.tile([128, C], F32, name="vps", tag="vps")
                    for dc in range(3):
                        nc.tensor.matmul(
                            ps[:], xT[dc][:, n32 * 128:(n32 + 1) * 128],
                            wch("Wv", dc),
                            start=(dc == 0), stop=(dc == 2))
                    nc.scalar.copy(v_all[:, n32 * C:(n32 + 1) * C], ps[:])

        # ---- attention (scores transposed; no max-subtraction needed) ----
        oT = [attin.tile([128, TNS], F32, name=f"oT{cc}", tag=f"oT{cc}")
              for cc in range(3)]
        with tc.tile_pool(name="spsum", bufs=3, space="PSUM") as spsum, \
             tc.tile_pool(name="opsum", bufs=1, space="PSUM") as opsum, \
             tc.tile_pool(name="dpsum", bufs=1, space="PSUM") as dpsum, \
             tc.tile_pool(name="epool", bufs=6) as epool, \
             tc.tile_pool(name="npool", bufs=2) as npool:
            for T in range(TNS // 512):
                o_ps = [opsum.tile([128, 512], F32, name=f"ops{cc}",
                                   tag=f"ops{cc}") for cc in range(3)]
                d_ps = dpsum.tile([1, 512], F32, name="dps", tag="dps")
                for n32 in range(32):
                    s_ps = spsum.tile([128, 512], F32, name="sps", tag="sps")
                    for cc in range(3):
                        nc.tensor.matmul(
                            s_ps[:], kT[cc][:, n32 * 128:(n32 + 1) * 128],
                            qT[:, cc * TNS + T * 512: cc * TNS + (T + 1) * 512],
                            start=(cc == 0), stop=(cc == 2))
                    e_t = epool.tile([128, 512], ve_st, name="e_t", tag="e_t")
                    nc.scalar.activation(e_t[:], s_ps[:], EXP, scale=SCALE)
                    for cc in range(3):
                        nc.tensor.matmul(
                            o_ps[cc][:],
                            v_all[:, n32 * C + cc * 128: n32 * C + (cc + 1) * 128],
                            e_t[:], start=(n32 == 0), stop=(n32 == 31))
                    nc.tensor.matmul(d_ps[:], ones_col[:], e_t[:],
                                     start=(n32 == 0), stop=(n32 == 31))
                rec = npool.tile([1, 512], F32, name="rec", tag="rec")
                nc.vector.reciprocal(rec[:], d_ps[:])
                b_ps = spsum.tile([128, 512], F32, name="bps", tag="sps")
                nc.tensor.matmul(b_ps[:], ones_row[:], rec[:],
                                 start=True, stop=True)
                rec_b = npool.tile([128, 512], F32, name="rec_b", tag="rec_b")
                nc.vector.tensor_copy(rec_b[:], b_ps[:])
                for cc in range(3):
                    nc.vector.tensor_mul(oT[cc][:, T * 512:(T + 1) * 512],
                                         o_ps[cc][:], rec_b[:])

        oT_d = dram.tile([C, TNS], F32, name=f"oT_d{rep}", tag="oT_d")
        for cc in range(3):
            nc.sync.dma_start(oT_d[cc * 128:(cc + 1) * 128, :], oT[cc][:])

    # ---- pair exchange + permutation-to-contiguous ----
    zbuf = dram.tile([TN, C], F32, name=f"zbuf{rep}", tag="zbuf")
    zview = zbuf[:].rearrange("a b -> (a b)").rearrange("(c t) -> c t", t=TN)
    if with_collective:
        gath = dram.tile([2 * C, TNS], F32, name=f"gath{rep}", tag="gath")
        nc.gpsimd.collective_compute(
            "AllGather", mybir.AluOpType.bypass,
            replica_groups=[[0, 1], [2, 3], [4, 5], [6, 7]],
            ins=[oT_d[:].opt()], outs=[gath[:].opt()])
        for g in range(2):
            nc.sync.dma_start(zview[:, g * TNS:(g + 1) * TNS],
                              gath[g * C:(g + 1) * C, :])
    else:
        for g in range(2):
            nc.sync.dma_start(zview[:, g * TNS:(g + 1) * TNS], oT_d[:])

    # ---- permuted output projection ----
    with tc.tile_pool(name="fpool", bufs=3) as fpool, \
         tc.tile_pool(name="fpsum", bufs=2, space="PSUM") as fpsum, \
         tc.tile_pool(name="ftpsum", bufs=2, space="PSUM") as ftpsum:
        for it in range(TN // 128):
            r_t = fpool.tile([128, C], F32, name="r_t", tag="r_t")
            nc.sync.dma_start(r_t[:], zbuf[it * 128:(it + 1) * 128, :])
            op_ch = fpool.tile([128, 3 * 128], qk_st, name="op_ch", tag="op_ch")
            for jc in range(3):
                p_tr = ftpsum.tile([128, 128], F32, name="p_tr", tag="p_tr")
                nc.tensor.transpose(p_tr[:], r_t[:, jc * 128:(jc + 1) * 128],
                                    ident[:])
                nc.vector.tensor_copy(op_ch[:, jc * 128:(jc + 1) * 128], p_tr[:])
            out_ps = fpsum.tile([128, C], F32, name="out_ps", tag="out_ps")
            for jc in range(3):
                nc.tensor.matmul(out_ps[:], op_ch[:, jc * 128:(jc + 1) * 128],
                                 wch("Wp", jc), start=(jc == 0), stop=(jc == 2))
            o_t = fpool.tile([128, C], F32, name="o_t", tag="o_t")
            nc.vector.tensor_add(o_t[:], out_ps[:], bias_b[:])
            nc.sync.dma_start(out_d[it * 128:(it + 1) * 128, :], o_t[:])


def make_in_maps(inputs):
    x = np.asarray(inputs["x"], np.float32)
    t = np.asarray(inputs["t"], np.float32)
    maps = []
    for p in range(N_CORES):
        b, h = p // 2, p % 2
        maps.append({
            "x": np.ascontiguousarray(x[b]),
            "t": np.ascontiguousarray(t[b, h * TNS:(h + 1) * TNS]),
            "Wq": np.asarray(inputs["Wq"], np.float32),
            "Wk": np.asarray(inputs["Wk"], np.float32),
            "Wv": np.asarray(inputs["Wv"], np.float32),
            "Wp": np.asarray(inputs["Wp"], np.float32),
            "bp": np.asarray(inputs["bp"], np.float32).reshape(1, C),
        })
    return maps


def assemble(results):
    out = np.empty((B, TN, C), np.float32)
    for p in range(N_CORES):
        b, h = p // 2, p % 2
        out[b, h * TNS:(h + 1) * TNS] = results[p]["out"][h * TNS:(h + 1) * TNS]
    return out


_NC_CACHE = {}


def _get_nc(repeat=1):
    key = repeat
    if key not in _NC_CACHE:
        _NC_CACHE[key] = build(repeat=repeat)
    return _NC_CACHE[key]


def kernel(**inputs) -> np.ndarray:
    nc = _get_nc()
    in_maps = make_in_maps(inputs)
    res = run_bass_kernel_spmd(nc, in_maps, list(range(N_CORES)))
    return assemble(res.results)

